# revision 8
# baseline (speedup 1.0000x reference)
"""Distributed sparse embedding lookup (mean combiner) on 8 Trainium2 cores.

Strategy (data-parallel over output rows; fp16 table replicated per core):
  - Each core owns 1/8 of the output rows (13312). row_indices is sorted,
    so each core's keys are a contiguous slice of the input.
  - The embedding table is uploaded once as fp16 padded to a 256 B row
    stride ([1M, 128] fp16, data in the first 64 columns). A raw
    dma_gather with elem_size=64 (128 B descriptors, 256 B stride  --
    HW-validated) fetches each key's row: half the DMA cost of fp32.
    int16 gather indices force 32768-row vocab windows: one gather
    instruction per window (31), keys ordered window-major.
  - dma_scatter_add (DRAM destination, fp16, elem_size=64 = 128 B
    descriptors at 256 B stride -- HW-validated) accumulates the
    rows directly into a padded fp16 accumulator in DRAM, which IS the
    kernel output. Duplicate targets within one scatter instruction lose
    updates (HW-verified), so the stream is arranged so every ~3072-key
    contiguous chunk is duplicate-free; rows with too many keys in one
    window get extra "overflow" accumulator slots that the host merges.
  - The accumulator's first 64 columns are zeroed through the same SWDGE
    queue before the scatters (FIFO ring order guarantees ordering).
  - Host converts the fp16 accumulator to fp32, adds overflow slots,
    multiplies by 1/count (the mean), and reshapes. Error is fp16 table
    rounding + fp16 accumulation (~5e-4 rel).
"""
import numpy as np

_B, _S, _D = 4096, 26, 64
_V = 1_000_000
_M = 8
_R = _B * _S              # 106496 output rows
_RC = _R // _M            # 13312 rows per core
_WIN = 32768
_NWIN = (_V + _WIN - 1) // _WIN      # 31
_CSL = 24                 # scatter chunk size in slots (24*128 = 3072 idx)
_GMAX = 4096              # max num_idxs per gather instruction (HW-validated)

_prog_cache = {}


def _cdiv(a, b):
    return (a + b - 1) // b


def _pack16(v, budget, pad):
    out = np.full(budget, pad, dtype=np.int16)
    out[: len(v)] = v
    return np.tile(out.reshape(-1, 16).T, (8, 1))


def _prep_core(keys, rows, slots_w, starts, NT):
    """Order one core's (key, row) stream window-major (on the SHARED slot
    geometry) such that every _CSL-slot contiguous chunk is duplicate-free
    in scatter targets. Returns per-core stream tensors + merge map.
    """
    counts = np.bincount(rows, minlength=_RC)
    invc_row = (1.0 / np.maximum(counts, 1.0)).astype(np.float32)

    win = keys // _WIN
    order = np.lexsort((rows, win))
    k_s, r_s, w_s = keys[order], rows[order], win[order]
    wb = np.searchsorted(w_s, np.arange(_NWIN + 1))
    n_chunks = _cdiv(NT, _CSL)

    # chunk id of a global slot
    def chunk_of(slot):
        return slot // _CSL

    # per-window portions: list of (chunk, capacity_keys)
    # plus per-chunk used-target sets
    used = [set() for _ in range(n_chunks)]
    overflow_of = {}          # row -> list of overflow acc slots
    merge_map = []            # (acc_slot, row)
    next_ov = [_RC]           # next overflow slot (row _RC reserved for dump below)

    DUMP = None  # assigned after overflow count known; use sentinel -1 now

    # output per window: for each portion, list of (key_rel, target, invc)
    stream_key = np.zeros(NT * 128, np.int64)
    stream_tgt = np.full(NT * 128, -1, np.int64)   # -1 => pad (dump)

    for w in range(_NWIN):
        lo, hi = int(wb[w]), int(wb[w + 1])
        if lo == hi:
            continue
        kk = k_s[lo:hi] - w * _WIN
        rr = r_s[lo:hi]
        s0, s1 = int(starts[w]), int(starts[w + 1])
        # portions: split window slot range at chunk boundaries
        bounds = [s0]
        c = chunk_of(s0)
        while (c + 1) * _CSL < s1:
            bounds.append((c + 1) * _CSL)
            c += 1
        bounds.append(s1)
        portions = []   # (chunk_id, pos_start, capacity)
        for i in range(len(bounds) - 1):
            a, b = bounds[i], bounds[i + 1]
            if b > a:
                portions.append([chunk_of(a), a * 128, (b - a) * 128, 0])
                # [chunk, base_pos, capacity, fill]

        # group same-row keys
        ro = np.argsort(rr, kind="stable")
        kk, rr = kk[ro], rr[ro]
        grp_bounds = np.flatnonzero(np.r_[True, rr[1:] != rr[:-1], True])
        # place constrained groups first (larger groups first)
        groups = [(int(rr[grp_bounds[i]]), grp_bounds[i], grp_bounds[i + 1])
                  for i in range(len(grp_bounds) - 1)]
        groups.sort(key=lambda g: g[1] - g[2])  # descending size

        for row, a, b in groups:
            kcnt = b - a
            for j in range(kcnt):
                key_rel = int(kk[a + j])
                placed = False
                # try: each portion x (primary target, then overflows)
                tgts = [row] + overflow_of.get(row, [])
                for t in tgts:
                    for p in portions:
                        if p[3] >= p[2]:
                            continue
                        if (row, t) in used[p[0]]:
                            continue
                        pos = p[1] + p[3]
                        p[3] += 1
                        used[p[0]].add((row, t))
                        stream_key[pos] = key_rel
                        stream_tgt[pos] = t
                        placed = True
                        break
                    if placed:
                        break
                if not placed:
                    # allocate a new overflow slot for this row
                    t = next_ov[0]
                    next_ov[0] += 1
                    overflow_of.setdefault(row, []).append(t)
                    merge_map.append((t, row))
                    ok = False
                    for p in portions:
                        if p[3] >= p[2]:
                            continue
                        if (row, t) in used[p[0]]:
                            continue
                        pos = p[1] + p[3]
                        p[3] += 1
                        used[p[0]].add((row, t))
                        stream_key[pos] = key_rel
                        stream_tgt[pos] = t
                        ok = True
                        break
                    if not ok:
                        raise RuntimeError("portion capacity exhausted")
        # remaining positions in portions stay pads (target dump, key 0)

    EX = next_ov[0] - _RC          # overflow slots used
    DUMP = _RC + EX                # dump row index
    NR = _RC + EX + 1              # acc rows (before rounding)
    stream_tgt[stream_tgt < 0] = DUMP

    # verify: no (chunk, target) duplicate except DUMP
    for c in range(n_chunks):
        a, b = c * _CSL * 128, min((c + 1) * _CSL * 128, NT * 128)
        t = stream_tgt[a:b]
        t = t[t != DUMP]
        assert len(np.unique(t)) == len(t), f"dup in chunk {c}"

    # gather idx cols: [128, NT*8] int16; window-relative keys, pads = 0
    gidx = _pack16(stream_key.astype(np.int16), NT * 128, np.int16(0))
    sidx = _pack16(stream_tgt.astype(np.int16), NT * 128, np.int16(DUMP))
    return {
        "gidx": np.ascontiguousarray(gidx),
        "sidx": np.ascontiguousarray(sidx),
        "invc_row": invc_row,
        "NR": NR,
        "EX": EX,
        "DUMP": DUMP,
        "merge": merge_map,
    }


def _prep(values, row_indices):
    values = np.asarray(values).astype(np.int64)
    row_indices = np.asarray(row_indices).astype(np.int64)
    if np.any(np.diff(row_indices) < 0):
        order = np.argsort(row_indices, kind="stable")
        values, row_indices = values[order], row_indices[order]
    bounds = np.searchsorted(row_indices, np.arange(_M + 1) * _RC)
    per_core = []
    for c in range(_M):
        lo, hi = bounds[c], bounds[c + 1]
        per_core.append((values[lo:hi], row_indices[lo:hi] - c * _RC))
    # shared slot geometry: max per-window slot count over cores
    slots_w = [0] * _NWIN
    for kk, _rr in per_core:
        wcnt = np.bincount(kk // _WIN, minlength=_NWIN)
        for w in range(_NWIN):
            slots_w[w] = max(slots_w[w], _cdiv(int(wcnt[w]), 128))
    starts = np.cumsum([0] + slots_w)
    NT = int(starts[-1])
    n_chunks = _cdiv(NT, _CSL)
    chunks = [(c * _CSL, min((c + 1) * _CSL, NT)) for c in range(n_chunks)]
    cores = [_prep_core(kk, rr, slots_w, starts, NT) for kk, rr in per_core]
    NR = _cdiv(max(cc["NR"] for cc in cores), 128) * 128
    return cores, slots_w, starts, NT, NR, chunks


def _build_shared(slots_w, starts, NT, NR, chunks, n_reps=1):
    from concourse import bacc, mybir, tile

    fp16 = mybir.dt.float16
    i16 = mybir.dt.int16

    nc = bacc.Bacc(None, target_bir_lowering=False, debug=False,
                   num_swdge_queues=1)
    tbl = nc.dram_tensor("table", [_V, 128], fp16, kind="ExternalInput")
    gidx_d = nc.dram_tensor("gidx", [128, NT * 8], i16, kind="ExternalInput")
    sidx_d = nc.dram_tensor("sidx", [128, NT * 8], i16, kind="ExternalInput")
    acc_d = nc.dram_tensor("out", [NR, 128], fp16, kind="ExternalOutput")

    ZS = _cdiv(NR, 128)

    def raw_gather(out_ap, in_ap, idxs_ap, num_idxs):
        g = nc.gpsimd
        stride_bytes = 128 * 2
        _in_ap = g.lower_ap_dma(in_ap, for_custom_bir_dma=True)
        _idxs_ap = g.lower_ap(idxs_ap)
        _out_ap = g.lower_ap(out_ap)
        return g.add_instruction(
            mybir.InstDMAGatherAnt(
                name=g.bass.get_next_instruction_name(),
                ins=[*_in_ap, _idxs_ap, g.lower_val_access(g.to_reg(num_idxs))],
                outs=[_out_ap],
                transpose=False,
                num_idxs=num_idxs,
                elem_size=_D,
                stride_bytes_256=stride_bytes // 256,
                gen_mode=0,
                single_packet=False,
                queue_num=0,
            )
        )

    with tile.TileContext(nc) as tc:
        with (
            tc.tile_pool(name="meta", bufs=1) as mpool,
            tc.tile_pool(name="data", bufs=1) as dpool,
        ):
            gix = mpool.tile([128, NT * 8], i16, tag="gix")
            six = mpool.tile([128, NT * 8], i16, tag="six")
            nc.sync.dma_start(out=gix[:], in_=gidx_d[:])
            nc.sync.dma_start(out=six[:], in_=sidx_d[:])
            zsrc = mpool.tile([128, ZS, _D], fp16, tag="zsrc")
            nc.vector.memset(zsrc[:], 0.0)

            gat = dpool.tile([128, NT, _D], fp16, tag="gat")

            for _rep in range(n_reps):
                # zero the acc first (queue-0 FIFO orders it before the
                # scatters, which also run on queue 0)
                ZSPL = _cdiv(ZS, 3)
                za = 0
                while za < ZS:
                    zb = min(za + ZSPL, ZS)
                    nc.gpsimd.dma_start(
                        out=acc_d[za * 128:zb * 128, 0:_D],
                        in_=zsrc[:, za:zb, :],
                    )
                    za = zb

                # interleave gathers (one per window) with scatters: chunk c
                # fires as soon as all its slots are gathered, so the FIFO
                # ring overlaps gather and scatter traffic.
                ci = 0
                for w in range(_NWIN):
                    s0, s1 = int(starts[w]), int(starts[w + 1])
                    base = w * _WIN
                    wsize = min(_WIN, _V - base)
                    sa = s0
                    while sa < s1:
                        sb = min(sa + _GMAX // 128, s1)
                        nkk = (sb - sa) * 128
                        raw_gather(
                            out_ap=gat[:, sa:sb, :],
                            in_ap=tbl[base:base + wsize, 0:_D],
                            idxs_ap=gix[:, sa * 8:sb * 8],
                            num_idxs=nkk,
                        )
                        sa = sb
                    while ci < len(chunks) and chunks[ci][1] <= s1:
                        c0, c1 = chunks[ci]
                        nck = (c1 - c0) * 128
                        nc.gpsimd.dma_scatter_add(
                            out_ap=acc_d[:, 0:_D],
                            in_ap=gat[:, c0:c1, :],
                            idxs_ap=six[:, c0 * 8:c1 * 8],
                            num_idxs=nck,
                            num_idxs_reg=nck,
                            elem_size=_D,
                            elem_step=128,
                            queue_num=0,
                            single_packet=False,
                        )
                        ci += 1
                assert ci == len(chunks)
    nc.compile()
    return nc


def _get_table(emb_table):
    t = np.asarray(emb_table, dtype=np.float32)
    pad = np.zeros((_V, 128), np.float16)
    pad[:, :_D] = t.astype(np.float16)
    return pad


def _state(values, row_indices, emb_table, n_reps=1):
    cores, slots_w, starts, NT, NR, chunks = _prep(values, row_indices)
    key = (tuple(slots_w), NT, NR, tuple(chunks), n_reps)
    if key not in _prog_cache:
        _prog_cache[key] = _build_shared(slots_w, starts, NT, NR, chunks,
                                         n_reps=n_reps)
    nc = _prog_cache[key]
    tblpad = _get_table(emb_table)
    in_maps = []
    for cc in cores:
        in_maps.append({
            "table": tblpad,
            "gidx": cc["gidx"],
            "sidx": cc["sidx"],
        })
    return nc, in_maps, cores


def kernel(values, row_indices, emb_table):
    from concourse.bass_utils import run_bass_kernel_spmd

    nc, in_maps, cores = _state(values, row_indices, emb_table)
    res = run_bass_kernel_spmd(nc, in_maps, core_ids=list(range(_M)))
    parts = []
    for c in range(_M):
        acc = np.asarray(res.results[c]["out"]).astype(np.float32)
        out_c = acc[:_RC, :_D].copy()
        mg = cores[c]["merge"]
        if mg:
            slots = np.array([m[0] for m in mg])
            rws = np.array([m[1] for m in mg])
            np.add.at(out_c, rws, acc[slots, :_D])
        out_c *= cores[c]["invc_row"][:, None]
        parts.append(out_c)
    full = np.concatenate(parts, axis=0)
    return np.ascontiguousarray(full.reshape(_B, _S, _D).astype(np.float32))
